# revision 9
# baseline (speedup 1.0000x reference)
"""Trainium2 Bass kernel for the arm-sampling rollout problem.

Math: the reference's 2048-step scan x <- x - (A@x)*dt with
A = P diag(exp(D)) P^-1 has the closed form
    hidden[k] = P diag(lam_i^k) P^-1 x0,   lam_i = 1 - dt*exp(D_i)
so actions^T[ch, k] = tanh(sum_i G[ch,i] * c_i * lam_i^k + bm[ch]) with
G = Wm @ P and c = P^-1 x0 (solved on-device by unpivoted Gauss-Jordan —
P is well-conditioned for this problem family). The output is then the
memory-bound broadcast  out[arm, j] = 150*eps[arm, j] + 15000*act_flat[j]
over a [5000, 4096] array, sharded 625 arms per core across 8 cores.

DMA layout: the bulk eps/out stream owns the sync HWDGE queue with
16KB-per-partition packets; every small prologue load is a natural
contiguous DMA on the gpsimd queue, transposed/broadcast on-chip via PE.
"""

import numpy as np

import concourse.bass as bass
import concourse.bacc as bacc
import concourse.mybir as mybir
import concourse.tile as tile
from concourse.bass_utils import run_bass_kernel_spmd

N_ARMS = 5000
N_STEPS = 2048
H = 10
F = 2 * N_STEPS  # 4096 flattened per-arm elements
N_CORES = 8
ARMS_PER_CORE = N_ARMS // N_CORES  # 625
FP = mybir.dt.float32

_NC_CACHE: dict = {}


def build_nc():
    AFT = mybir.ActivationFunctionType
    ALU = mybir.AluOpType

    nc = bacc.Bacc(
        "TRN2",
        target_bir_lowering=False,
        debug=False,
        enable_asserts=True,
        num_devices=N_CORES,
    )

    eps_d = nc.dram_tensor("eps", [ARMS_PER_CORE, F], FP, kind="ExternalInput")
    tgt_d = nc.dram_tensor("target", [2], FP, kind="ExternalInput")
    D_d = nc.dram_tensor("D", [H], FP, kind="ExternalInput")
    P_d = nc.dram_tensor("P", [H, H], FP, kind="ExternalInput")
    W1_d = nc.dram_tensor("W1", [256, 2], FP, kind="ExternalInput")
    b1_d = nc.dram_tensor("b1", [256], FP, kind="ExternalInput")
    W2_d = nc.dram_tensor("W2", [H, 256], FP, kind="ExternalInput")
    b2_d = nc.dram_tensor("b2", [H], FP, kind="ExternalInput")
    Wm_d = nc.dram_tensor("Wm", [2, H], FP, kind="ExternalInput")
    bm_d = nc.dram_tensor("bm", [2], FP, kind="ExternalInput")
    out_d = nc.dram_tensor("out", [ARMS_PER_CORE, F], FP, kind="ExternalOutput")

    with tile.TileContext(nc) as tc:
        with (
            tc.tile_pool(name="sbc", bufs=1) as sbc,
            tc.tile_pool(name="sbgj", bufs=2) as sbgj,
            tc.tile_pool(name="sbeps", bufs=5) as sbeps,
            tc.tile_pool(name="psa", bufs=2, space=bass.MemorySpace.PSUM) as psa,
            tc.tile_pool(name="psbc", bufs=2, space=bass.MemorySpace.PSUM) as psbc,
            tc.tile_pool(name="psact", bufs=2, space=bass.MemorySpace.PSUM) as psact,
            tc.tile_pool(name="psB", bufs=2, space=bass.MemorySpace.PSUM) as psB,
        ):
            # ---------- small constant loads: contiguous, gpsimd queue --------
            w1n0 = sbc.tile([128, 2], FP, tag="w1n0")
            nc.sync.dma_start(w1n0[:], W1_d.ap()[0:128, :])
            w1n1 = sbc.tile([128, 2], FP, tag="w1n1")
            nc.sync.dma_start(w1n1[:], W1_d.ap()[128:256, :])
            b1n = sbc.tile([1, 256], FP, tag="b1n")
            nc.sync.dma_start(b1n[:], b1_d.ap()[None, :])
            tgtr = sbc.tile([1, 2], FP, tag="tgtr")
            nc.sync.dma_start(tgtr[:], tgt_d.ap()[None, :])
            w2n = sbc.tile([H, 256], FP, tag="w2n")
            nc.sync.dma_start(w2n[:], W2_d.ap())
            p_sb = sbc.tile([H, H], FP, tag="p_sb")
            nc.sync.dma_start(p_sb[:], P_d.ap())
            wmT = sbc.tile([H, 2], FP, tag="wmT")
            nc.sync.dma_start(wmT[:], Wm_d.ap().rearrange("m k -> k m"))
            b2s = sbc.tile([H, 1], FP, tag="b2s")
            nc.sync.dma_start(b2s[:], b2_d.ap()[:, None])
            bm0 = sbc.tile([1, 1], FP, tag="bm0")
            nc.sync.dma_start(bm0[:], bm_d.ap()[0:1][:, None])
            bm1 = sbc.tile([1, 1], FP, tag="bm1")
            nc.sync.dma_start(bm1[:], bm_d.ap()[1:2][:, None])
            ds = sbc.tile([H, 1], FP, tag="ds")
            nc.sync.dma_start(ds[:], D_d.ap()[:, None])
            aug = sbgj.tile([H, H + 1], FP, tag="aug")
            nc.sync.dma_start(aug[:, 0:H], P_d.ap())

            # ---------- bulk loads first, spread across engine DGE queues -----
            # Each engine's dma_start lands on its own DGE queue; one queue
            # caps well below the HBM port, so round-robin in/out halves
            # across five queues for concurrency.
            qeng = [nc.sync, nc.scalar]
            NSPL = 4
            HF = F // NSPL
            eps_tiles = []
            rows = list(range(0, ARMS_PER_CORE, 128))
            qi = 0
            for r in rows:
                pt = min(128, ARMS_PER_CORE - r)
                t = sbeps.tile([128, F], FP, tag="eps")
                for h in range(NSPL):
                    qeng[qi % 2].dma_start(
                        t[0:pt, h * HF : (h + 1) * HF],
                        eps_d.ap()[r : r + pt, h * HF : (h + 1) * HF],
                    )
                    qi += 1
                eps_tiles.append((t, r, pt))

            ones = sbc.tile([1, 128], FP, tag="ones")
            nc.vector.memset(ones[:], 1.0)

            # idm[p, j] = 1 if p == j (via iota p-j then ==0)
            idi = sbc.tile([H, H], mybir.dt.int32, tag="idi")
            nc.gpsimd.iota(idi[:], pattern=[[-1, H]], base=0, channel_multiplier=1)
            idm = sbc.tile([H, H], FP, tag="idm")
            nc.vector.tensor_scalar(idm[:], idi[:], 0, None, ALU.is_equal)
            # oht[:, 10k:10k+10] is the matrix with row k all-ones: used as
            # matmul lhsT to broadcast row k of aug to every partition.
            oht = sbc.tile([H, H * H], FP, tag="oht")
            oht3 = oht[:].rearrange("p (k r) -> p k r", r=H)
            for r in range(H):
                nc.vector.tensor_copy(oht3[:, :, r : r + 1], idm[:, :, None])

            # ---------- on-chip transposes / broadcasts of the small consts ---
            # tb[p, k] = target[k] on every partition (ones-matmul broadcast)
            tbp = psa.tile([128, 2], FP, tag="mm")
            nc.tensor.matmul(tbp[:], ones[:], tgtr[:])
            tb = sbc.tile([128, 2], FP, tag="tb")
            nc.vector.tensor_copy(tb[:], tbp[:])
            # b1 halves as [128, 1] columns (PE transpose, identity = ones[0:1,0:1])
            b1p0 = psa.tile([128, 1], FP, tag="mm")
            nc.tensor.matmul(
                b1p0[:], b1n[0:1, 0:128], ones[0:1, 0:1], is_transpose=True
            )
            b1a = sbc.tile([128, 1], FP, tag="b1a")
            nc.vector.tensor_copy(b1a[:], b1p0[:])
            b1p1 = psa.tile([128, 1], FP, tag="mm")
            nc.tensor.matmul(
                b1p1[:], b1n[0:1, 128:256], ones[0:1, 0:1], is_transpose=True
            )
            b1b = sbc.tile([128, 1], FP, tag="b1b")
            nc.vector.tensor_copy(b1b[:], b1p1[:])
            # W2^T halves [128, 10] (PE transpose with identity idm)
            w2tp0 = psa.tile([128, H], FP, tag="mm")
            nc.tensor.matmul(w2tp0[:], w2n[:, 0:128], idm[:], is_transpose=True)
            w2t0 = sbc.tile([128, H], FP, tag="w2t0")
            nc.vector.tensor_copy(w2t0[:], w2tp0[:])
            w2tp1 = psa.tile([128, H], FP, tag="mm")
            nc.tensor.matmul(w2tp1[:], w2n[:, 128:256], idm[:], is_transpose=True)
            w2t1 = sbc.tile([128, H], FP, tag="w2t1")
            nc.vector.tensor_copy(w2t1[:], w2tp1[:])

            # ---------- x0 = W2 @ relu(W1 @ target + b1) + b2 ----------
            # W1 @ target on DVE: hp = w1[:,0]*t0 + w1[:,1]*t1 (per-partition)
            u0 = sbc.tile([128, 1], FP, tag="u0")
            nc.vector.tensor_scalar_mul(u0[:], w1n0[:, 1:2], tb[:, 1:2])
            hp0 = sbc.tile([128, 1], FP, tag="hp0")
            nc.vector.scalar_tensor_tensor(
                hp0[:], w1n0[:, 0:1], tb[:, 0:1], u0[:], ALU.mult, ALU.add
            )
            h0 = sbc.tile([128, 1], FP, tag="h0")
            nc.scalar.activation(h0[:], hp0[:], AFT.Relu, bias=b1a[:], scale=1.0)
            u1 = sbc.tile([128, 1], FP, tag="u1")
            nc.vector.tensor_scalar_mul(u1[:], w1n1[:, 1:2], tb[:, 1:2])
            hp1 = sbc.tile([128, 1], FP, tag="hp1")
            nc.vector.scalar_tensor_tensor(
                hp1[:], w1n1[:, 0:1], tb[:, 0:1], u1[:], ALU.mult, ALU.add
            )
            h1 = sbc.tile([128, 1], FP, tag="h1")
            nc.scalar.activation(h1[:], hp1[:], AFT.Relu, bias=b1b[:], scale=1.0)
            x0p = psa.tile([H, 1], FP, tag="mm")
            nc.tensor.matmul(x0p[:], w2t0[:], h0[:], start=True, stop=False)
            nc.tensor.matmul(x0p[:], w2t1[:], h1[:], start=False, stop=True)
            x0s = sbc.tile([H, 1], FP, tag="x0s")
            nc.scalar.activation(x0s[:], x0p[:], AFT.Identity, bias=b2s[:], scale=1.0)

            # ---------- lam = 1 - 0.01*exp(D); lnlam ----------
            es = sbc.tile([H, 1], FP, tag="es")
            nc.scalar.activation(es[:], ds[:], AFT.Exp)
            lam = sbc.tile([H, 1], FP, tag="lam")
            nc.vector.tensor_scalar(lam[:], es[:], -0.01, 1.0, ALU.mult, ALU.add)
            lnl = sbc.tile([H, 1], FP, tag="lnl")
            nc.scalar.activation(lnl[:], lam[:], AFT.Ln)

            # ---------- Gauss-Jordan solve P c = x0 (no pivoting) ----------
            nc.vector.tensor_copy(aug[:, H : H + 1], x0s[:])
            for k in range(H):
                bc = psbc.tile([H, H + 1], FP, tag="bc")
                nc.tensor.matmul(bc[:], oht[:, H * k : H * k + H], aug[:])
                piv = sbgj.tile([H, 1], FP, tag="piv")
                nc.vector.reciprocal(piv[:], bc[:, k : k + 1])
                S = sbgj.tile([H, H + 1], FP, tag="S")
                nc.vector.tensor_scalar_mul(S[:], bc[:], piv[:])
                fn = sbgj.tile([H, 1], FP, tag="fn")
                nc.vector.tensor_sub(fn[:], idm[:, k : k + 1], aug[:, k : k + 1])
                aug2 = sbgj.tile([H, H + 1], FP, tag="aug")
                nc.vector.scalar_tensor_tensor(
                    aug2[:], S[:], fn[:], aug[:], ALU.mult, ALU.add
                )
                aug = aug2

            # ---------- G^T = (Wm @ P)^T via matmul(lhsT=P, rhs=Wm^T) ----------
            gtp = psa.tile([H, 2], FP, tag="mm")
            nc.tensor.matmul(gtp[:], p_sb[:], wmT[:])
            gts = sbc.tile([H, 2], FP, tag="gts")
            nc.vector.tensor_scalar_mul(gts[:], gtp[:], aug[:, H : H + 1])

            # ---------- Vc[i, k] = c_i * lam_i^k ----------
            ki = sbc.tile([H, N_STEPS], mybir.dt.int32, tag="ki")
            nc.gpsimd.iota(ki[:], pattern=[[1, N_STEPS]], base=0, channel_multiplier=0)
            kf = sbc.tile([H, N_STEPS], FP, tag="kf")
            nc.vector.tensor_copy(kf[:], ki[:])
            vc = sbc.tile([H, N_STEPS], FP, tag="vc")
            nc.scalar.activation(vc[:], kf[:], AFT.Exp, scale=lnl[:])

            # ---------- actions: per-channel rows on partition 0 ----------
            # ats[0, 0:2048] = tanh-row ch0, ats[0, 2048:4096] = ch1
            ats = sbc.tile([1, F], FP, tag="ats")
            for ch in range(2):
                bmt = bm0 if ch == 0 else bm1
                for j in range(N_STEPS // 512):
                    atp = psact.tile([1, 512], FP, tag="actT")
                    nc.tensor.matmul(
                        atp[:], gts[:, ch : ch + 1], vc[:, 512 * j : 512 * (j + 1)]
                    )
                    nc.scalar.activation(
                        ats[:, ch * N_STEPS + 512 * j : ch * N_STEPS + 512 * (j + 1)],
                        atp[:],
                        AFT.Tanh,
                        bias=bmt[:],
                        scale=1.0,
                    )

            # ---------- B[p, 2t+ch] = 15000 * ats[ch, t], broadcast to 128 ----
            Bsb = sbc.tile([128, F], FP, tag="B")
            B3 = Bsb[:].rearrange("p (t m) -> p t m", m=2)
            for ch in range(2):
                for j in range(N_STEPS // 512):
                    bp = psB.tile([128, 512], FP, tag="B")
                    nc.tensor.matmul(
                        bp[:],
                        ones[:],
                        ats[:, ch * N_STEPS + 512 * j : ch * N_STEPS + 512 * (j + 1)],
                    )
                    nc.scalar.activation(
                        B3[:, 512 * j : 512 * (j + 1), ch : ch + 1],
                        bp[:, :, None],
                        AFT.Copy,
                        scale=15000.0,
                    )

            # ---------- main loop: out = 150*eps + B ----------
            qo = 1
            for t, r, pt in eps_tiles:
                nc.vector.scalar_tensor_tensor(
                    t[0:pt, :], t[0:pt, :], 150.0, Bsb[0:pt, :], ALU.mult, ALU.add
                )
                for h in range(NSPL):
                    qeng[qo % 2].dma_start(
                        out_d.ap()[r : r + pt, h * HF : (h + 1) * HF],
                        t[0:pt, h * HF : (h + 1) * HF],
                    )
                    qo += 1

    nc.compile()
    return nc


def get_nc():
    if "nc" not in _NC_CACHE:
        _NC_CACHE["nc"] = build_nc()
    return _NC_CACHE["nc"]


def kernel(**inputs):
    nc = get_nc()
    eps = np.ascontiguousarray(
        np.asarray(inputs["eps"], dtype=np.float32).reshape(N_ARMS, F)
    )
    small = {
        k: np.ascontiguousarray(np.asarray(inputs[k], dtype=np.float32))
        for k in ["target", "D", "P", "W1", "b1", "W2", "b2", "Wm", "bm"]
    }
    in_maps = [
        {**small, "eps": eps[i * ARMS_PER_CORE : (i + 1) * ARMS_PER_CORE]}
        for i in range(N_CORES)
    ]
    res = run_bass_kernel_spmd(nc, in_maps, core_ids=list(range(N_CORES)))
    out = np.concatenate([res.results[i]["out"] for i in range(N_CORES)], axis=0)
    return out.reshape(N_ARMS, 2, N_STEPS)
